# revision 13
# baseline (speedup 1.0000x reference)
"""AppUsageFEDformer Trainium2 kernel — 8-core data-parallel Bass implementation.

Strategy: pure data parallelism over batch (64 -> 8 per NeuronCore).  Each
core runs the full model on its batch shard:
  embedding gather -> 2x encoder layer (Q-proj, Fourier block via DFT
  matmuls, mode mix, iDFT, out-proj, series-decomp as banded matmul, FFN)
  -> final my_layernorm -> vocab projection.

Layouts per core (b = local batch 0..7, l = seq 0..511, tokens t = b*512+l):
  x_S  (spine, f32):  SBUF [128(l%128), (b, c=l//128, d)]  "S layout"
  x_T  (bf16):        SBUF [128(d%128), (dc=d//128, t)]    "T layout",
                      built via cast-DMA to DRAM + xbar DMA-transpose.
All matmuls run in bf16 (f32 PSUM accumulate); the f32 spine only ever
receives f32 adds/subtracts, so precision is dominated by the bf16
rounding of branch outputs, each of which is small relative to x.
All weights are pre-arranged on the host into stationary/moving layouts so
device code only does contiguous DMA loads.
"""

import os

import numpy as np
import ml_dtypes

import concourse.bass as bass
import concourse.tile as tile
from concourse import mybir
from concourse.bass_utils import run_bass_kernel_spmd

F32 = mybir.dt.float32
BF16 = mybir.dt.bfloat16
I32 = mybir.dt.int32
I16 = mybir.dt.int16
AF = mybir.ActivationFunctionType
ALU = mybir.AluOpType

B, L, D, H, DFF, MODES, ELAYERS = 64, 512, 512, 8, 2048, 32, 2
VOCAB, NUM_APP, KERNEL = 10000, 10000, 25
E = D // H  # 64
NCORES = 8
BL = B // NCORES  # 8 local batch
NT = BL * L       # 4096 local tokens
bf16 = ml_dtypes.bfloat16


# ---------------------------------------------------------------- host prep

def _movavg_matrix():
    """M[l_src, l_out]: weight of x[l_src] in moving_avg[l_out], including
    edge replication (pad (K-1)//2 each side with edge values)."""
    M = np.zeros((L, L), np.float64)
    pad = (KERNEL - 1) // 2
    for lo in range(L):
        for j in range(lo - pad, lo + pad + 1):
            M[min(max(j, 0), L - 1), lo] += 1.0 / KERNEL
    return M.astype(np.float32)


def _dft_c4():
    """C4 [128, (4 c, 64)]: stationary for DFT.  col j<32: cos(2pi*l*j/L);
    j>=32: -sin(2pi*l*(j-32)/L) with l = c*128+p."""
    out = np.zeros((128, 4, 64), np.float32)
    for c in range(4):
        lv = c * 128 + np.arange(128)
        for m in range(MODES):
            ang = 2.0 * np.pi * lv * m / L
            out[:, c, m] = np.cos(ang)
            out[:, c, 32 + m] = -np.sin(ang)
    return out.astype(bf16)


def _idft_d():
    """Dstack [64, 512]: rows m<32: sc(m)*cos(2pi*m*l'/L); rows 32+m:
    -sc(m)*sin(...), sc = (2-delta_m0)/L."""
    out = np.zeros((64, L), np.float32)
    lp = np.arange(L)
    for m in range(MODES):
        sc = (1.0 if m == 0 else 2.0) / L
        ang = 2.0 * np.pi * m * lp / L
        out[m] = sc * np.cos(ang)
        out[32 + m] = -sc * np.sin(ang)
    return out.astype(bf16)


def prep_weights(inp):
    """Pre-arrange all weights into SBUF-shaped host arrays."""
    w = {}
    w["embt"] = (np.asarray(inp["app_emb_w"], np.float32)
                 + np.asarray(inp["time_b"], np.float32)[None, :])
    w["tw_rep"] = np.broadcast_to(
        np.asarray(inp["time_w"], np.float32), (128, D)).copy()
    w["c4"] = _dft_c4()
    w["dstack"] = _idft_d()

    M = _movavg_matrix()
    adiag = np.zeros((128, 4, 128), np.float32)
    for c in range(4):
        adiag[:, c, :] = M[c * 128:(c + 1) * 128, c * 128:(c + 1) * 128]
    w["adiag"] = adiag.astype(bf16)
    # band tiles zero-padded to 32/64-aligned partition bases (matmul
    # requires base_partition in {0, 32, 64} matching on both operands)
    aup = np.zeros((32, 3, 128), np.float32)    # tile (co+1, co): rows 0:12
    alo = np.zeros((128, 3, 128), np.float32)   # tile (co-1, co): rows 116:128
    for co in range(3):
        aup[0:12, co, :] = M[(co + 1) * 128:(co + 1) * 128 + 12,
                             co * 128:(co + 1) * 128]
    for co in range(1, 4):
        alo[116:128, co - 1, :] = M[co * 128 - 12:co * 128,
                                    co * 128:(co + 1) * 128]
    w["aup"] = aup.astype(bf16)
    w["alo"] = alo.astype(bf16)

    Wq = np.asarray(inp["Wq"], np.float32)
    Wo = np.asarray(inp["Wo"], np.float32)
    wr = np.asarray(inp["four_wr"], np.float32)
    wi = np.asarray(inp["four_wi"], np.float32)
    c1 = np.asarray(inp["conv1_w"], np.float32)
    c2 = np.asarray(inp["conv2_w"], np.float32)
    bq = np.asarray(inp["bq"], np.float32)
    bo = np.asarray(inp["bo"], np.float32)

    for l in range(ELAYERS):
        wqt = np.zeros((128, 4, D), np.float32)
        for k in range(4):
            wqt[:, k, :] = Wq[l].T[k * 128:(k + 1) * 128, :]
        w[f"wqt{l}"] = wqt.astype(bf16)
        w[f"bq{l}"] = np.broadcast_to(bq[l], (128, D)).copy()
        w[f"bo{l}"] = np.broadcast_to(bo[l], (128, D)).copy()

        wc = np.zeros((128, H, MODES, 128), np.float32)
        for h in range(H):
            for m in range(MODES):
                wc[0:64, h, m, 0:64] = wr[l, h, :, :, m]     # i x o
                wc[0:64, h, m, 64:128] = wi[l, h, :, :, m]
                wc[64:128, h, m, 0:64] = -wi[l, h, :, :, m]
                wc[64:128, h, m, 64:128] = wr[l, h, :, :, m]
        w[f"wc{l}"] = wc.astype(bf16)

        wot = np.zeros((128, 4, D), np.float32)
        for jc in range(4):
            wot[:, jc, :] = Wo[l].T[jc * 128:(jc + 1) * 128, :]
        w[f"wot{l}"] = wot.astype(bf16)

        c1s = np.zeros((128, 4, 16, 128), np.float32)
        for dc in range(4):
            for ft in range(16):
                c1s[:, dc, ft, :] = c1[l][ft * 128:(ft + 1) * 128,
                                          dc * 128:(dc + 1) * 128].T
        w[f"c1{l}"] = c1s.astype(bf16)

        c2t = np.zeros((128, 16, D), np.float32)
        for fc in range(16):
            c2t[:, fc, :] = c2[l].T[fc * 128:(fc + 1) * 128, :]
        w[f"c2{l}"] = c2t.astype(bf16)

    w["normw"] = np.broadcast_to(
        np.asarray(inp["norm_w"], np.float32), (128, D)).copy()
    pw = np.asarray(inp["proj_w"], np.float32)       # [10000, 536]
    pwt = np.zeros((128, 4, NUM_APP), np.float32)
    for ck in range(4):
        pwt[:, ck, :] = pw.T[ck * 128:(ck + 1) * 128, :]
    w["pwt"] = pwt.astype(bf16)
    w["pwt4"] = pw.T[512:536, :].astype(bf16)        # [24, 10000]
    w["pb"] = np.broadcast_to(
        np.asarray(inp["proj_b"], np.float32), (8, NUM_APP)).copy()
    w["ones"] = np.ones((128, 1), np.float32).astype(bf16)
    w["ident8"] = np.eye(8, dtype=np.float32)
    return w


# ---------------------------------------------------------------- builder

def _decomp(nc, p_ps, p_xbf, x_S, adiag_t, aup_t, alo_t):
    """x_S <- x_S - moving_avg(x_S) via banded bf16 matmul over l.
    A per-b bf16 copy of x_S serves as the matmul moving operand."""
    for b in range(BL):
        x_bf = p_xbf.tile([128, 4, D], BF16, tag="xbf")
        for c in range(4):
            nc.vector.tensor_copy(x_bf[:, c, :], x_S[:, b, c, :])
        for co in range(4):
            mms = [(adiag_t[:, co, :], x_bf[:, co, :])]
            if co < 3:
                mms.append((aup_t[:, co, :], x_bf[0:32, co + 1, :]))
            if co > 0:
                mms.append((alo_t[64:128, co - 1, :],
                            x_bf[64:128, co - 1, :]))
            ps_a = p_ps.tile([128, D], F32, tag="ps")
            for i, (lhsT, rhs) in enumerate(mms):
                nc.tensor.matmul(ps_a[:], lhsT, rhs,
                                 start=(i == 0), stop=(i == len(mms) - 1))
            nc.vector.tensor_sub(x_S[:, b, co, :], x_S[:, b, co, :], ps_a[:])


def build_nc(num_devices=NCORES, debug=False):
    nc = bass.Bass("TRN2", target_bir_lowering=False, debug=False,
                   num_devices=num_devices)
    P = {}

    def param(name, shape, dtype):
        P[name] = nc.declare_dram_parameter(name, list(shape), dtype,
                                            isOutput=False)

    param("x_app", [BL, L], I32)
    param("x_time", [BL, L], F32)
    param("tv_last", [BL, 24], F32)
    param("embt", [VOCAB, D], F32)
    param("tw_rep", [128, D], F32)
    param("c4", [128, 4, 64], BF16)
    param("dstack", [64, L], BF16)
    param("adiag", [128, 4, 128], BF16)
    param("aup", [32, 3, 128], BF16)
    param("alo", [128, 3, 128], BF16)
    for l in range(ELAYERS):
        param(f"wqt{l}", [128, 4, D], BF16)
        param(f"bq{l}", [128, D], F32)
        param(f"bo{l}", [128, D], F32)
        param(f"wc{l}", [128, H, MODES, 128], BF16)
        param(f"wot{l}", [128, 4, D], BF16)
        param(f"c1{l}", [128, 4, 16, 128], BF16)
        param(f"c2{l}", [128, 16, D], BF16)
    param("normw", [128, D], F32)
    param("pwt", [128, 4, NUM_APP], BF16)
    param("pwt4", [24, NUM_APP], BF16)
    param("pb", [8, NUM_APP], F32)
    param("ones", [128, 1], BF16)
    param("ident8", [8, 8], F32)

    OUT = nc.declare_dram_parameter("out", [BL, NUM_APP], F32, isOutput=True)
    xrow = nc.dram_tensor("xrow", [NT, D], BF16)   # transpose bounce

    dbg = {}

    def dbg_dump(name, ap_or_tile, shape, dtype=F32):
        if debug:
            dbg[name] = nc.declare_dram_parameter(
                "dbg_" + name, list(shape), dtype, isOutput=True)
            nc.sync.dma_start(dbg[name].ap(), ap_or_tile)

    with tile.TileContext(nc) as tc:
        with tc.tile_pool(name="spine", bufs=1) as p_spine, \
             tc.tile_pool(name="bigT", bufs=1) as p_bigT, \
             tc.tile_pool(name="xbf", bufs=2) as p_xbf, \
             tc.tile_pool(name="qsb", bufs=1) as p_qsb, \
             tc.tile_pool(name="four", bufs=1) as p_four, \
             tc.tile_pool(name="wcb", bufs=1) as p_wcb, \
             tc.tile_pool(name="wts", bufs=1) as p_wts, \
             tc.tile_pool(name="wly", bufs=1) as p_wly, \
             tc.tile_pool(name="sml", bufs=1) as p_sml, \
             tc.tile_pool(name="pwp", bufs=1) as p_pw, \
             tc.tile_pool(name="ps", bufs=4, space="PSUM") as p_ps, \
             tc.tile_pool(name="ps2", bufs=2, space="PSUM") as p_ps2:

            # ---------------- constants / shared weights ----------------
            c4_t = p_wts.tile([128, 4, 64], BF16, tag="c4")
            nc.sync.dma_start(c4_t[:], P["c4"][:])
            dst_t = p_wts.tile([64, L], BF16, tag="dstack")
            nc.sync.dma_start(dst_t[:], P["dstack"][:])
            adiag_t = p_wts.tile([128, 4, 128], BF16, tag="adiag")
            nc.sync.dma_start(adiag_t[:], P["adiag"][:])
            aup_t = p_wts.tile([32, 3, 128], BF16, tag="aup")
            nc.sync.dma_start(aup_t[:], P["aup"][:])
            alo_t = p_wts.tile([128, 3, 128], BF16, tag="alo")
            nc.sync.dma_start(alo_t[:], P["alo"][:])
            tw_t = p_wts.tile([128, D], F32, tag="twrep")
            nc.sync.dma_start(tw_t[:], P["tw_rep"][:])
            ones_t = p_wts.tile([128, 1], BF16, tag="ones")
            nc.sync.dma_start(ones_t[:], P["ones"][:])
            id8_t = p_wts.tile([8, 8], F32, tag="id8")
            nc.sync.dma_start(id8_t[:], P["ident8"][:])
            normw_t = p_wts.tile([128, D], F32, tag="normw")
            nc.sync.dma_start(normw_t[:], P["normw"][:])

            # ---------------- embedding ----------------
            idx_sb = p_sml.tile([128, 32], I32, tag="idxsb")
            # idx_sb[p, b*4+c] = x_app[b, c*128+p]  (= token t's app id)
            nc.sync.dma_start(
                idx_sb[:],
                P["x_app"].ap().rearrange("b (c p) -> p (b c)", p=128))

            x_S = p_spine.tile([128, BL, 4, D], F32, tag="spine")
            for b in range(BL):
                for c in range(4):
                    j = b * 4 + c
                    nc.gpsimd.indirect_dma_start(
                        out=x_S[:, b, c, :], out_offset=None,
                        in_=P["embt"][:],
                        in_offset=bass.IndirectOffsetOnAxis(
                            ap=idx_sb[:, j:j + 1], axis=0))

            xt_t = p_sml.tile([128, 32], F32, tag="xt")
            nc.sync.dma_start(
                xt_t[:],
                P["x_time"].ap().rearrange("b (c p) -> p (b c)", p=128))
            for b in range(BL):
                for c in range(4):
                    j = b * 4 + c
                    nc.vector.scalar_tensor_tensor(
                        out=x_S[:, b, c, :], in0=tw_t[:],
                        scalar=xt_t[:, j:j + 1], in1=x_S[:, b, c, :],
                        op0=ALU.mult, op1=ALU.add)

            dbg_dump("x0", x_S[:], [128, BL, 4, D])

            # ---------------- layers ----------------
            for l in range(ELAYERS):
                wqt_t = p_wly.tile([128, 4, D], BF16, tag="wqt")
                nc.sync.dma_start(wqt_t[:], P[f"wqt{l}"][:])
                bq_t = p_wly.tile([128, D], F32, tag="bq")
                nc.sync.dma_start(bq_t[:], P[f"bq{l}"][:])
                bo_t = p_wly.tile([128, D], F32, tag="bo")
                nc.sync.dma_start(bo_t[:], P[f"bo{l}"][:])
                wot_t = p_wly.tile([128, 4, D], BF16, tag="wot")
                nc.sync.dma_start(wot_t[:], P[f"wot{l}"][:])

                # ---- x_T via cast-DMA to DRAM + xbar transpose ----
                for b in range(BL):
                    for c in range(4):
                        t0 = (b * 4 + c) * 128
                        nc.gpsimd.dma_start(xrow[t0:t0 + 128, :],
                                            x_S[:, b, c, :])
                x_T = p_bigT.tile([128, 4, NT], BF16, tag="bigT")
                for k in range(4):
                    nc.sync.dma_start_transpose(
                        x_T[:, k, :], xrow[:, k * 128:(k + 1) * 128])

                # ---- Q-proj + DFT (per b) ----
                xs_sb = p_four.tile([64, BL, 2, H, 32], BF16, tag="xs")
                for b in range(BL):
                    q_sb = p_qsb.tile([128, 4, D], BF16, tag="qsb")
                    for c in range(4):
                        t0 = (b * 4 + c) * 128
                        ps_q = p_ps.tile([128, D], F32, tag="ps")
                        for k in range(4):
                            nc.tensor.matmul(
                                ps_q[:], x_T[:, k, t0:t0 + 128],
                                wqt_t[:, k, :],
                                start=(k == 0), stop=(k == 3))
                        nc.vector.tensor_add(q_sb[:, c, :], ps_q[:], bq_t[:])
                    ps_xs = p_ps2.tile([64, D], F32, tag="psxs")
                    for c in range(4):
                        nc.tensor.matmul(ps_xs[:], c4_t[:, c, :],
                                         q_sb[:, c, :],
                                         start=(c == 0), stop=(c == 3))
                    # psum free order (h, q, i32) -> xs_sb[:, b, q, h, i32]
                    nc.vector.tensor_copy(
                        xs_sb[:, b, :, :, :].rearrange("p q h i -> p h q i"),
                        ps_xs[:].rearrange("p (h q i) -> p h q i", h=8, q=2))

                # ---- shuffle1: xs -> xsT [(ri,i), (b, h, m)] ----
                xsT = p_qsb.tile([128, BL, H, 32], BF16, tag="qsb")
                for b in range(BL):
                    for ri in range(2):
                        for q in range(2):
                            r0 = ri * 64 + q * 32
                            nc.vector.transpose(
                                xsT[r0:r0 + 32, b, :, :]
                                .rearrange("p h m -> p (h m)"),
                                xs_sb[ri * 32:ri * 32 + 32, b, q, :, :]
                                .rearrange("p h i -> p (h i)"))

                # ---- mode mix ----
                # psum tile (mh, hp): [128, 512]; rows 32g+b (g = h//2);
                # cols mi*128 + (ri*64 + o).  modes_sb rows sparse 32g+b.
                modes_sb = p_four.tile([128, 2, 2, 64, MODES], BF16,
                                       tag="modes")
                nc.gpsimd.memset(modes_sb[:], 0.0)
                for mh in range(8):
                    m0 = mh * 4
                    wc_t = p_wcb.tile([128, H, 4, 128], BF16, tag="wcb")
                    nc.sync.dma_start(
                        wc_t[:], P[f"wc{l}"][:, :, m0:m0 + 4, :])
                    for hp in range(2):
                        ps_mm = p_ps.tile([128, D], F32, tag="ps")
                        for g in range(4):
                            h = g * 2 + hp
                            for mi in range(4):
                                nc.tensor.matmul(
                                    ps_mm[32 * g:32 * g + 8,
                                          mi * 128:(mi + 1) * 128],
                                    xsT[:, :, h, m0 + mi],
                                    wc_t[:, h, mi, :],
                                    start=True, stop=True,
                                    tile_position=(0, 32 * g))
                        for g in range(4):
                            nc.vector.tensor_copy(
                                modes_sb[32 * g:32 * g + 8, hp, :, :,
                                         m0:m0 + 4]
                                .rearrange("p r o m -> p m r o"),
                                ps_mm[32 * g:32 * g + 8, :]
                                .rearrange("p (m r o) -> p m r o",
                                           m=4, r=2))

                # ---- shuffle2: per-g stream transpose into a scratch with
                # 32-wide padded b, then compact the 8 valid cols ----
                modes_T = p_four.tile([64, 2, 64, 32], BF16, tag="modesT")
                mscr = p_four.tile([64, 2, 64, 32], BF16, tag="xs")
                for g in range(4):
                    for ri in range(2):
                        for hp in range(2):
                            nc.vector.transpose(
                                mscr[ri * 32:(ri + 1) * 32, hp, :, :]
                                .rearrange("p o w -> p (o w)"),
                                modes_sb[32 * g:32 * (g + 1), hp, ri, :, :]
                                .rearrange("p o m -> p (o m)"))
                    nc.vector.tensor_copy(
                        modes_T[:, :, :, g * 8:(g + 1) * 8],
                        mscr[:, :, :, 0:8])

                # ---- iDFT -> y_S [l', (lc, b, he)] ----
                y_S = p_bigT.tile([128, 4, BL, D], BF16, tag="bigT")
                mt5 = modes_T[:].rearrange("p hp o (g bb) -> p g hp o bb",
                                           g=4)
                for lc in range(4):
                    for b in range(BL):
                        ps_y = p_ps.tile([128, D], F32, tag="ps")
                        nc.tensor.matmul(
                            ps_y[:], dst_t[:, lc * 128:(lc + 1) * 128],
                            mt5[:, :, :, :, b],
                            start=True, stop=True)
                        nc.vector.tensor_copy(y_S[:, lc, b, :], ps_y[:])

                # ---- Wo + bo + residual (faithful reshape semantics:
                #      new_x[b, l=(h,e), d'] = sum_j y[b, h, e, j] Wo[d', j])
                for b in range(BL):
                    for ht in range(4):
                        ps_wo = p_ps.tile([128, D], F32, tag="ps")
                        for jc in range(4):
                            nc.tensor.matmul(
                                ps_wo[:],
                                y_S[:, jc, b, ht * 128:(ht + 1) * 128],
                                wot_t[:, jc, :],
                                start=(jc == 0), stop=(jc == 3))
                        nc.vector.tensor_add(ps_wo[:], ps_wo[:], bo_t[:])
                        nc.vector.tensor_add(x_S[:, b, ht, :],
                                             x_S[:, b, ht, :], ps_wo[:])

                # ---- decomp1 ----
                _decomp(nc, p_ps, p_xbf, x_S, adiag_t, aup_t, alo_t)

                if l == 0:
                    dbg_dump("res1", x_S[:], [128, BL, 4, D])

                # ---- res1_T ----
                for b in range(BL):
                    for c in range(4):
                        t0 = (b * 4 + c) * 128
                        nc.gpsimd.dma_start(xrow[t0:t0 + 128, :],
                                            x_S[:, b, c, :])
                r1T = p_bigT.tile([128, 4, NT], BF16, tag="bigT")
                for k in range(4):
                    nc.sync.dma_start_transpose(
                        r1T[:, k, :], xrow[:, k * 128:(k + 1) * 128])

                # ---- FFN (two f-half passes, partial z-adds) ----
                for fh in range(2):
                    c1_t = p_wly.tile([128, 4, 8, 128], BF16, tag="c1")
                    nc.sync.dma_start(
                        c1_t[:], P[f"c1{l}"][:, :, fh * 8:(fh + 1) * 8, :])
                    c2_t = p_wly.tile([128, 8, D], BF16, tag="c2")
                    nc.sync.dma_start(
                        c2_t[:], P[f"c2{l}"][:, fh * 8:(fh + 1) * 8, :])
                    for b in range(BL):
                        y1_t = p_wcb.tile([128, 8, D], BF16, tag="wcb")
                        for ft in range(8):
                            ps_f1 = p_ps.tile([128, D], F32, tag="ps")
                            for dc in range(4):
                                nc.tensor.matmul(
                                    ps_f1[:], c1_t[:, dc, ft, :],
                                    r1T[:, dc, b * 512:(b + 1) * 512],
                                    start=(dc == 0), stop=(dc == 3))
                            nc.scalar.activation(y1_t[:, ft, :], ps_f1[:],
                                                 AF.Relu)
                        for ht in range(4):
                            ps_f2 = p_ps.tile([128, D], F32, tag="ps")
                            for fc in range(8):
                                nc.tensor.matmul(
                                    ps_f2[:],
                                    y1_t[:, fc, ht * 128:(ht + 1) * 128],
                                    c2_t[:, fc, :],
                                    start=(fc == 0), stop=(fc == 7))
                            nc.vector.tensor_add(x_S[:, b, ht, :],
                                                 x_S[:, b, ht, :], ps_f2[:])

                # ---- decomp2 ----
                _decomp(nc, p_ps, p_xbf, x_S, adiag_t, aup_t, alo_t)

            dbg_dump("xfin", x_S[:], [128, BL, 4, D])

            # ---------------- final layernorm + projection ----------------
            sum_x = p_sml.tile([128, 32], F32, tag="sumx")
            nc.vector.tensor_reduce(
                sum_x[:], x_S[:].rearrange("p b c d -> p (b c) d"),
                mybir.AxisListType.X, ALU.add)
            sq_scratch = p_sml.tile([128, D], BF16, tag="sqs")
            sum_sq = p_sml.tile([128, 32], F32, tag="sumsq")
            for j in range(32):
                b, c = j // 4, j % 4
                nc.scalar.activation(sq_scratch[:], x_S[:, b, c, :],
                                     AF.Square,
                                     accum_out=sum_sq[:, j:j + 1])
            mu = p_sml.tile([128, 32], F32, tag="mu")
            nc.vector.tensor_scalar_mul(mu[:], sum_x[:], 1.0 / D)
            var = p_sml.tile([128, 32], F32, tag="var")
            nc.vector.tensor_mul(var[:], mu[:], mu[:])
            nc.vector.scalar_tensor_tensor(
                out=var[:], in0=sum_sq[:], scalar=1.0 / D, in1=var[:],
                op0=ALU.mult, op1=ALU.subtract)
            nc.vector.tensor_scalar_add(var[:], var[:], 1e-5)
            std = p_sml.tile([128, 32], F32, tag="std")
            nc.scalar.activation(std[:], var[:], AF.Sqrt)
            rstd = p_sml.tile([128, 32], F32, tag="rstd")
            nc.vector.reciprocal(rstd[:], std[:])

            # xh0 = (x - mu) * rstd   (bf16; affine w folded at the end,
            # bias b cancels in xh[last] - mean_l(xh))
            xh0 = p_bigT.tile([128, 4, BL, D], BF16, tag="bigT")
            for j in range(32):
                b, c = j // 4, j % 4
                nc.vector.tensor_scalar(
                    xh0[:, c, b, :], x_S[:, b, c, :],
                    mu[:, j:j + 1], rstd[:, j:j + 1],
                    ALU.subtract, ALU.mult)

            last_out = p_sml.tile([8, 536], F32, tag="lastout")
            for b in range(BL):
                ps_m = p_ps.tile([1, D], F32, tag="ps")
                for c in range(4):
                    nc.tensor.matmul(ps_m[:], ones_t[:], xh0[:, c, b, :],
                                     start=(c == 0), stop=(c == 3))
                # engine APs must start at partition 0/32/64/96; move the
                # last-token row (partition 127) and the per-b output row
                # via small SBUF-to-SBUF DMAs instead.
                xlast = p_sml.tile([1, D], BF16, tag="xlast")
                nc.sync.dma_start(xlast[:], xh0[127:128, 3, b, :])
                diff = p_sml.tile([1, D], F32, tag="diff")
                nc.vector.scalar_tensor_tensor(
                    out=diff[:], in0=ps_m[:], scalar=-1.0 / L,
                    in1=xlast[:], op0=ALU.mult, op1=ALU.add)
                row = p_sml.tile([1, D], F32, tag="row")
                nc.vector.tensor_mul(row[:], diff[:], normw_t[0:1, :])
                nc.sync.dma_start(last_out[b:b + 1, 0:D], row[:])
            nc.sync.dma_start(last_out[:, D:D + 24], P["tv_last"][:])

            # transpose last_out -> lastT (bf16 for the projection matmul)
            lastT = p_sml.tile([128, 4, 8], BF16, tag="lastT")
            for ck in range(4):
                ps_t = p_ps.tile([128, 8], F32, tag="ps")
                nc.tensor.transpose(ps_t[:],
                                    last_out[:, ck * 128:(ck + 1) * 128],
                                    id8_t[:])
                nc.vector.tensor_copy(lastT[:, ck, :], ps_t[:])
            lastT4 = p_sml.tile([24, 8], BF16, tag="lastT4")
            ps_t4 = p_ps.tile([24, 8], F32, tag="ps")
            nc.tensor.transpose(ps_t4[:], last_out[:, 512:536], id8_t[:])
            nc.vector.tensor_copy(lastT4[:], ps_t4[:])

            # projection, streamed over 20 vocab slices
            for vs in range(20):
                v0 = vs * 512
                vw = min(512, NUM_APP - v0)
                pw_t = p_wcb.tile([128, 4, 512], BF16, tag="wcb")
                nc.sync.dma_start(pw_t[:, :, 0:vw], P["pwt"][:, :, v0:v0 + vw])
                pw4_t = p_pw.tile([24, 512], BF16, tag="pw4")
                nc.sync.dma_start(pw4_t[:, 0:vw], P["pwt4"][:, v0:v0 + vw])
                pb_t = p_pw.tile([8, 512], F32, tag="pb")
                nc.sync.dma_start(pb_t[:, 0:vw], P["pb"][:, v0:v0 + vw])
                ps_p = p_ps2.tile([8, 512], F32, tag="psxs")
                for ck in range(4):
                    nc.tensor.matmul(ps_p[:, 0:vw], lastT[:, ck, :],
                                     pw_t[:, ck, 0:vw],
                                     start=(ck == 0), stop=False)
                nc.tensor.matmul(ps_p[:, 0:vw], lastT4[:], pw4_t[:, 0:vw],
                                 start=False, stop=True)
                sc_t = p_pw.tile([8, 512], F32, tag="sc")
                nc.vector.tensor_add(sc_t[:, 0:vw], ps_p[:, 0:vw],
                                     pb_t[:, 0:vw])
                nc.sync.dma_start(OUT[:, v0:v0 + vw], sc_t[:, 0:vw])

    return nc


# ---------------------------------------------------------------- runner

_CACHED = {}


def kernel(**inputs):
    import bir_legalize
    bir_legalize.install()

    x_app = np.asarray(inputs["x_app"])
    x_time = np.asarray(inputs["x_time"], np.float32)
    time_vecs = np.asarray(inputs["time_vecs"], np.float32)

    w = prep_weights(inputs)
    if "nc" not in _CACHED:
        _CACHED["nc"] = build_nc()
    nc = _CACHED["nc"]

    in_maps = []
    for core in range(NCORES):
        sl = slice(core * BL, (core + 1) * BL)
        m = {"x_app": np.ascontiguousarray(x_app[sl]).astype(np.int32),
             "x_time": np.ascontiguousarray(x_time[sl]),
             "tv_last": np.ascontiguousarray(time_vecs[sl, L - 1, :])}
        m.update(w)
        in_maps.append(m)

    res = run_bass_kernel_spmd(nc, in_maps, list(range(NCORES)))
    out = np.concatenate([res.results[i]["out"] for i in range(NCORES)],
                         axis=0)
    return out.astype(np.float32)


if __name__ == "__main__":
    import reference
    inp = {k: np.asarray(v) for k, v in reference.setup_inputs().items()}
    got = kernel(**inp)
    exp = np.asarray(reference.reference(**reference.setup_inputs()))
    err = np.linalg.norm(got - exp) / np.linalg.norm(exp)
    print("Relative error:", err)
